# revision 43
# baseline (speedup 1.0000x reference)
"""Trainium2 Bass kernel for EnhancedGradedLoss (Huber + pairwise hinge ranking).

Algorithm (see reference): loss = 0.7 * SmoothL1(p, t) + 0.3 * ranking, where
ranking averages relu(1 - sign(t_i - t_j) * (p_i - p_j)) over i<j pairs with
t_i != t_j.

Device strategy (8 NeuronCores, SPMD):
  * Targets take a small discrete set of grades. Sort items by grade on host
    (O(n) prep). Every unordered pair (a, b) with grade(a) > grade(b)
    contributes relu(1 - p_a + p_b); equal-grade pairs contribute nothing.
  * For each grade level h below the top, "window h" pairs every row with
    grade > h against the columns of grade h. Rows are sharded across the 8
    cores (padded with dead rows that contribute exactly zero); the sorted
    prediction vector B is replicated to all 128 SBUF partitions per core
    via a stride-0 broadcast DMA (bf16).
  * Each [128 rows x n_h cols] tile is ONE fused instruction:
      - ScalarE: activation(Relu, bias=c_row, accum_out) -> sum relu(B + c)
      - VectorE: tensor_scalar(max, scalar1=-c_row, accum_out) at 4x bf16
        -> sum max(B, -c) == sum relu(B + c) - n_h * c   (host adds n_h * c)
    Work is split across both engines to balance their modeled busy time.
  * Huber = 0.5*d^2 - 0.5*relu(d-1)^2 - 0.5*relu(-d-1)^2, sharded 1/8 per
    core; VectorE preps d/relu terms, ScalarE squares+accumulates. This runs
    during the broadcast-DMA head so it is effectively free.
  * Raw Bass program (no Tile framework): explicit per-engine instruction
    streams with hand-placed semaphores; per-core differences are carried
    entirely by input data, so one SPMD program serves all 8 cores.
  * Device outputs are per-partition accumulators only ([128, ~16] per
    core); the host reduces them in float64 and applies the closed-form
    dead-row/max-trick corrections.
"""

import functools
import sys

import ml_dtypes
import numpy as np

sys.path.insert(0, "/opt/trn_rl_repo")

import concourse.bacc as bacc
import concourse.bass as bass
from concourse import mybir
from concourse.bass_utils import run_bass_kernel_spmd

ALPHA = 0.7
BETA = 0.3
W0_CHUNKS = 2  # window-0 broadcast head split (earlier compute start)
NCORES = 8
P = 128  # SBUF partitions


def _plan(targets_f, predictions_f):
    """Host-side planning: sort by grade, window layout, per-core row shards."""
    n = targets_f.shape[0]
    order = np.argsort(targets_f, kind="stable")
    ts = targets_f[order]
    ps = predictions_f[order].astype(np.float32)

    # grade level boundaries (targets take a small discrete set of values)
    levels, counts = np.unique(ts, return_counts=True)
    K = len(levels)
    offs = np.concatenate([[0], np.cumsum(counts)]).astype(np.int64)  # len K+1

    pmax = float(np.max(np.abs(ps))) if n else 0.0
    dead = -float(np.float32(np.ceil(pmax) + 2.0))

    # columns: all grades except the top one
    L = int(offs[K - 1]) if K >= 2 else 0
    bcols = ps[:L].copy()

    # c values for every sorted row: c = 1 - p  (float32 exactly as device uses)
    c_all = (np.float32(1.0) - ps).astype(np.float32)

    # windows: h = 0..K-2; cols = [offs[h], offs[h+1]); rows = positions >= offs[h+1]
    windows = []
    for h in range(K - 1):
        col0 = int(offs[h])
        ncol = int(offs[h + 1] - offs[h])
        row0 = int(offs[h + 1])
        m = n - row0
        if ncol == 0 or m == 0:
            continue
        q = -(-m // NCORES)  # ceil: rows per core
        t = -(-q // P)  # tiles per core
        windows.append(dict(col0=col0, ncol=ncol, row0=row0, m=m, q=q, T=t))

    # per-core row-constant arrays (window-major, each window padded to T*128)
    cp_cores = []
    for c in range(NCORES):
        parts = []
        for w in windows:
            r0 = w["row0"] + c * w["q"]
            r1 = min(w["row0"] + min((c + 1) * w["q"], w["m"]), n)
            r0 = min(r0, r1)
            vals = c_all[r0:r1]
            padded = np.full(w["T"] * P, dead, dtype=np.float32)
            padded[: len(vals)] = vals
            parts.append(padded)
        cp_cores.append(
            np.concatenate(parts) if parts else np.zeros(0, dtype=np.float32)
        )

    # engine assignment per (window, tile): balance modeled busy-ns.
    # Iterate in window (data-arrival) order so each engine's early tiles
    # come from the earliest-DMA'd window and neither engine stalls on a
    # later window's broadcast.
    tiles = []
    for wi, w in enumerate(windows):
        for tj in range(w["T"]):
            tiles.append((wi, tj, w["ncol"]))
    t_dve = 0.0
    t_act = 0.0
    assign = {}
    for wi, tj, ncol in tiles:
        cost_d = (58.0 + ncol / 4.0) / 0.96 + 45.0
        cost_a = (224.0 + ncol) / 1.2 + 190.0
        if t_dve + cost_d <= t_act + cost_a:
            assign[(wi, tj)] = "dve"
            t_dve += cost_d
        else:
            assign[(wi, tj)] = "act"
            t_act += cost_a

    # huber shard sizes
    ch = -(-n // NCORES)  # per-core elems
    cht = -(-ch // P)  # free-dim cols of [128, cht] tile
    chp = cht * P

    nt = sum(w["T"] for w in windows)
    nacc = nt + 3
    # compact per-engine accumulator slots, in (window, tile) emission order
    slots = {}
    nd = na = 0
    for wi, w in enumerate(windows):
        for tj in range(w["T"]):
            if assign[(wi, tj)] == "dve":
                slots[(wi, tj)] = nd
                nd += 1
            else:
                slots[(wi, tj)] = na
                na += 1

    meta = dict(
        n=n,
        K=K,
        levels=levels,
        counts=counts.astype(np.int64),
        offs=offs,
        L=L,
        dead=dead,
        windows=windows,
        assign=assign,
        nt=nt,
        nacc=nacc,
        slots=slots,
        nd=nd,
        na=na,
        ch=ch,
        cht=cht,
        chp=chp,
        rt=int(cp_cores[0].shape[0]),
    )
    return meta, bcols, cp_cores, ps


def _shape_key(meta):
    wkey = tuple(
        (w["col0"], w["ncol"], w["T"]) for w in meta["windows"]
    )
    akey = tuple(sorted(meta["assign"].items()))
    return (meta["n"], meta["L"], meta["rt"], meta["cht"], wkey, akey)


@functools.lru_cache(maxsize=8)
def _build_program(key):
    """Raw Bass program (no TileContext): explicit per-engine streams and
    semaphores. Value-independent given the shape key."""
    n, L, rt, cht, wkey, akey = key
    assign = dict(akey)
    chp = cht * P
    nd = sum(1 for _, e in akey if e == "dve")
    na = sum(1 for _, e in akey if e == "act")

    nc = bacc.Bacc("TRN2", enable_partition_id=False)

    tcols = rt // P
    combw = 2 * tcols + 2 * cht  # [cp | cn | pred | targ], partition-major
    d_b = nc.dram_tensor("bcols", [max(L, 1)], mybir.dt.bfloat16, kind="ExternalInput")
    d_comb = nc.dram_tensor(
        "comb", [combw * P], mybir.dt.float32, kind="ExternalInput"
    )
    d_acc = nc.dram_tensor("acc", [P, max(nd, 1) + 1], mybir.dt.float32, kind="ExternalOutput")
    d_acc2 = nc.dram_tensor("acc2", [P, na + 3], mybir.dt.float32, kind="ExternalOutput")

    fp32 = mybir.dt.float32
    bf16 = mybir.dt.bfloat16
    Alu = mybir.AluOpType
    Act = mybir.ActivationFunctionType
    npf32 = fp32
    npbf16 = bf16

    maxncol = max((ncol for _, ncol, _ in wkey), default=1)
    nw = len(wkey)

    bt = nc.alloc_sbuf_tensor("bt", [P, max(L, 1)], npbf16)
    comb = nc.alloc_sbuf_tensor("comb_t", [P, combw], npf32)
    acc_d = nc.alloc_sbuf_tensor("acc_d", [P, max(nd, 1) + 1], npf32)
    acc_a = nc.alloc_sbuf_tensor("acc_a", [P, na + 3], npf32)
    n_scr_d = max(sum(1 for _, e in akey if e == "dve"), 1)
    n_scr_a = max(sum(1 for _, e in akey if e == "act"), 1)
    scr_ds = [
        nc.alloc_sbuf_tensor(f"scr_d{i}", [P, maxncol], npbf16)
        for i in range(n_scr_d)
    ]
    scr_as = [
        nc.alloc_sbuf_tensor(f"scr_a{i}", [P, maxncol], npbf16)
        for i in range(n_scr_a)
    ]
    hd = nc.alloc_sbuf_tensor("hd", [P, cht], npf32)
    hr1 = nc.alloc_sbuf_tensor("hr1", [P, cht], npf32)
    he = nc.alloc_sbuf_tensor("he", [P, cht], npf32)
    hr2 = nc.alloc_sbuf_tensor("hr2", [P, cht], npf32)
    hs = nc.alloc_sbuf_tensor("hs", [P, cht], npf32)
    hs1 = nc.alloc_sbuf_tensor("hs1", [P, cht], npf32)
    hs2 = nc.alloc_sbuf_tensor("hs2", [P, cht], npf32)

    s_comb = nc.alloc_semaphore("s_comb")
    s_cn = nc.alloc_semaphore("s_cn")
    s_w = [nc.alloc_semaphore(f"s_w{i}") for i in range(max(nw, 1))]
    s_w0s = [s_w[0]] + [nc.alloc_semaphore(f"s_w0c{j}") for j in range(1, 8)]
    s_hub = nc.alloc_semaphore("s_hub")
    s_dve = nc.alloc_semaphore("s_dve")
    s_act = nc.alloc_semaphore("s_act")
    s_dp = nc.alloc_semaphore("s_dp")
    s_ap = nc.alloc_semaphore("s_ap")
    s_out = nc.alloc_semaphore("s_out")

    pts = comb[:, 2 * tcols : 2 * tcols + cht]
    tts = comb[:, 2 * tcols + cht : 2 * tcols + 2 * cht]

    # per-engine tile worklists: (wi, col0, ncol, cidx)
    work_d, work_a = [], []
    colbase = 0
    for wi, (col0, ncol, T) in enumerate(wkey):
        for tj in range(T):
            item = (wi, col0, ncol, colbase + tj)
            (work_d if assign[(wi, tj)] == "dve" else work_a).append(item)
        colbase += T

    with nc.Block() as block:

        @block.sync
        def _(sync):
            sync.dma_start(
                out=comb[:, :], in_=d_comb[:].rearrange("(p t) -> p t", p=P)
            ).then_inc(s_comb, 16)
            if L > 0:
                col0, ncol, _T = wkey[0]
                kch = min(W0_CHUNKS, len(s_w0s)) if ncol >= 512 else 1
                bnds = [ncol * j // kch for j in range(kch + 1)]
                for j in range(kch):
                    o, c = bnds[j], bnds[j + 1] - bnds[j]
                    src = bass.AP(
                        tensor=d_b[:].tensor,
                        offset=col0 + o,
                        ap=[[0, P], [1, c]],
                    )
                    sync.dma_start(
                        out=bt[:, col0 + o : col0 + o + c], in_=src
                    ).then_inc(s_w0s[j], 16)
                for wi in range(1, nw):
                    wcol0, wncol, _T = wkey[wi]
                    src = bass.AP(
                        tensor=d_b[:].tensor, offset=wcol0, ap=[[0, P], [1, wncol]]
                    )
                    sync.dma_start(
                        out=bt[:, wcol0 : wcol0 + wncol], in_=src
                    ).then_inc(s_w[wi], 16)
            d_stage = 0  # staged out-DMA measured slower (ring overhead)
            a_stage = 0
            need = 32
            if d_stage > 0:
                sync.wait_ge(s_dp, 1)
                sync.dma_start(
                    out=d_acc[:, :d_stage], in_=acc_d[:, :d_stage]
                ).then_inc(s_out, 16)
                need += 16
            if a_stage > 0:
                sync.wait_ge(s_ap, 1)
                sync.dma_start(
                    out=d_acc2[:, :a_stage], in_=acc_a[:, :a_stage]
                ).then_inc(s_out, 16)
                need += 16
            sync.wait_ge(s_dve, 1)
            with nc.allow_non_contiguous_dma(reason="tiny tail accumulators"):
                sync.dma_start(
                    out=d_acc[:, d_stage:], in_=acc_d[:, d_stage:]
                ).then_inc(s_out, 16)
                sync.wait_ge(s_act, 1)
                sync.dma_start(
                    out=d_acc2[:, a_stage:], in_=acc_a[:, a_stage:]
                ).then_inc(s_out, 16)
            sync.wait_ge(s_out, need)



        @block.vector
        def _(vector):
            vector.wait_ge(s_comb, 16)
            # Huber elementwise prep first: fills the broadcast-DMA head and
            # unblocks ScalarE's squares early.
            vector.tensor_tensor(out=hd[:, :], in0=pts, in1=tts, op=Alu.subtract)
            vector.drain()
            vector.tensor_scalar(
                out=hr1[:, :], in0=hd[:, :], scalar1=1.0, scalar2=0.0,
                op0=Alu.subtract, op1=Alu.max,
            )
            vector.tensor_scalar(
                out=he[:, :], in0=hd[:, :], scalar1=-1.0, scalar2=1.0,
                op0=Alu.mult, op1=Alu.subtract,
            )
            vector.drain()
            vector.tensor_scalar(
                out=hr2[:, :], in0=he[:, :], scalar1=0.0, scalar2=None, op0=Alu.max,
            ).then_inc(s_hub, 1)
            w0_split = bool(work_d) and work_d[0][0] == 0 and wkey[0][1] >= 512
            kch = min(W0_CHUNKS, 8) if (L > 0 and wkey and wkey[0][1] >= 512) else 1
            last = None
            if nd == 0:
                last = vector.memset(acc_d[:, :], 0.0)
            elif not w0_split or kch == 1:
                last = vector.memset(acc_d[:, nd : nd + 1], 0.0)
            seen = set()
            for sl, (wi, col0, ncol, cidx) in enumerate(work_d):
                if sl == 0 and w0_split and kch > 1:
                    # window 0 arrives in kch chunk-DMAs; process the first
                    # tile chunk-by-chunk so compute starts as data lands.
                    # Chunks 1..kch-1 accumulate into the extra slot.
                    bnds = [ncol * j // kch for j in range(kch + 1)]
                    vector.wait_ge(s_w0s[0], 16)
                    vector.tensor_scalar(
                        out=scr_ds[sl][:, : bnds[1]],
                        in0=bt[:, col0 : col0 + bnds[1]],
                        scalar1=comb[:, tcols + cidx : tcols + cidx + 1],
                        scalar2=None,
                        op0=Alu.max,
                        op1=Alu.add,
                        accum_out=acc_d[:, sl : sl + 1],
                    )
                    if kch == 2:
                        vector.wait_ge(s_w0s[1], 16)
                        last = vector.tensor_scalar(
                            out=scr_ds[sl][:, bnds[1] : ncol],
                            in0=bt[:, col0 + bnds[1] : col0 + ncol],
                            scalar1=comb[:, tcols + cidx : tcols + cidx + 1],
                            scalar2=None,
                            op0=Alu.max,
                            op1=Alu.add,
                            accum_out=acc_d[:, nd : nd + 1],
                        )
                        seen.add(0)
                        continue
                    ex = nc.alloc_sbuf_tensor(f"exacc", [P, max(kch - 1, 1)], npf32)
                    for j in range(1, kch):
                        o, cw = bnds[j], bnds[j + 1] - bnds[j]
                        vector.wait_ge(s_w0s[j], 16)
                        last = vector.tensor_scalar(
                            out=scr_ds[sl][:, o : o + cw],
                            in0=bt[:, col0 + o : col0 + o + cw],
                            scalar1=comb[:, tcols + cidx : tcols + cidx + 1],
                            scalar2=None,
                            op0=Alu.max,
                            op1=Alu.add,
                            accum_out=ex[:, j - 1 : j],
                        )
                    # fold the chunk accums into the extra output slot
                    vector.drain()
                    last = vector.tensor_reduce(
                        out=acc_d[:, nd : nd + 1],
                        in_=ex[:, : kch - 1],
                        axis=mybir.AxisListType.X,
                        op=Alu.add,
                    )
                    seen.add(0)
                    continue
                if wi not in seen:
                    if wi == 0:
                        for j in range(kch):
                            vector.wait_ge(s_w0s[j], 16)
                    else:
                        vector.wait_ge(s_w[wi], 16)
                    seen.add(wi)
                last = vector.tensor_scalar(
                    out=scr_ds[sl][:, :ncol],
                    in0=bt[:, col0 : col0 + ncol],
                    scalar1=comb[:, tcols + cidx : tcols + cidx + 1],
                    scalar2=None,
                    op0=Alu.max,
                    op1=Alu.add,
                    accum_out=acc_d[:, sl : sl + 1],
                )
            last.then_inc(s_dve, 1)

        @block.scalar
        def _(act):
            act.wait_ge(s_hub, 1)
            act.activation(
                out=hs[:, :], in_=hd[:, :], func=Act.Square,
                accum_out=acc_a[:, na : na + 1],
            )
            act.activation(
                out=hs1[:, :], in_=hr1[:, :], func=Act.Square,
                accum_out=acc_a[:, na + 1 : na + 2],
            )
            last = act.activation(
                out=hs2[:, :], in_=hr2[:, :], func=Act.Square,
                accum_out=acc_a[:, na + 2 : na + 3],
            )
            kch_a = min(W0_CHUNKS, 8) if (L > 0 and wkey and wkey[0][1] >= 512) else 1
            seen = set()
            for sl, (wi, col0, ncol, cidx) in enumerate(work_a):
                if wi not in seen:
                    if wi == 0:
                        for j in range(kch_a):
                            act.wait_ge(s_w0s[j], 16)
                    else:
                        act.wait_ge(s_w[wi], 16)
                    seen.add(wi)
                last = act.activation(
                    out=scr_as[sl][:, :ncol],
                    in_=bt[:, col0 : col0 + ncol],
                    func=Act.Relu,
                    bias=comb[:, cidx : cidx + 1],
                    scale=1.0,
                    accum_out=acc_a[:, sl : sl + 1],
                )
            last.then_inc(s_act, 1)

    nc.finalize()
    return nc


def _make_inputs(meta, bcols, cp_cores, predictions, targets):
    n = meta["n"]
    chp = meta["chp"]
    cht = meta["cht"]
    L = meta["L"]
    rt = meta["rt"]
    in_maps = []
    b_in = np.ascontiguousarray(
        bcols if L > 0 else np.zeros(1, dtype=np.float32), dtype=ml_dtypes.bfloat16
    )
    for c in range(NCORES):
        pc = np.zeros(chp, dtype=np.float32)
        tc_ = np.zeros(chp, dtype=np.float32)
        lo = c * meta["ch"]
        hi = min((c + 1) * meta["ch"], n)
        if hi > lo:
            pc[: hi - lo] = predictions[lo:hi]
            tc_[: hi - lo] = targets[lo:hi]
        cp = cp_cores[c] if rt > 0 else np.zeros(0, dtype=np.float32)
        tcols = rt // P
        cols = []
        if tcols > 0:
            cols.append(cp.reshape(tcols, P).T)
            cols.append(-cp.reshape(tcols, P).T)
        cols.append(pc.reshape(cht, P).T)
        cols.append(tc_.reshape(cht, P).T)
        comb2d = np.concatenate(cols, axis=1).astype(np.float32)  # [128, combw]
        in_maps.append({"bcols": b_in, "comb": np.ascontiguousarray(comb2d.ravel())})
    return in_maps


def _gather(meta, cp_cores, results):
    """Combine per-core accumulators into the scalar loss (float64 host math)."""
    n = meta["n"]
    nt = meta["nt"]
    windows = meta["windows"]
    assign = meta["assign"]

    slots = meta["slots"]
    nd = meta["nd"]
    num = 0.0
    hub_a = hub_b = hub_c = 0.0
    for c in range(NCORES):
        acc = results[c]["acc"].astype(np.float64)
        acc2 = results[c]["acc2"].astype(np.float64)
        # hinge accumulators + DVE correction:  sum relu = accum + ncol * c_row
        colbase = 0
        for wi, w in enumerate(windows):
            for tj in range(w["T"]):
                sl = slots[(wi, tj)]
                if assign[(wi, tj)] == "dve":
                    num += acc[:, sl].sum()
                    if sl == 0:
                        num += acc[:, nd].sum()  # split-tile extra slot
                    rows = cp_cores[c][(colbase + tj) * P : (colbase + tj + 1) * P]
                    num += w["ncol"] * rows.astype(np.float64).sum()
                else:
                    num += acc2[:, sl].sum()
            colbase += w["T"]
        na = meta["na"]
        hub_a += acc2[:, na].sum()
        hub_b += acc2[:, na + 1].sum()
        hub_c += acc2[:, na + 2].sum()

    huber = 0.5 * (hub_a - hub_b - hub_c) / n

    counts = meta["counts"].astype(np.int64)
    csum = np.cumsum(counts)
    cnt = int(np.sum(counts[1:] * csum[:-1])) if len(counts) > 1 else 0
    if cnt > 0:
        ranking = num / float(np.float32(cnt))
    else:
        ranking = 0.0

    return np.float32(ALPHA * huber + BETA * ranking)


def _host_fallback(predictions, targets):
    """Safety net for input distributions the device plan is not built for
    (e.g. near-continuous targets). Exact O(n^2) evaluation, row-chunked."""
    p = predictions.astype(np.float64)
    t = targets.astype(np.float64)
    n = len(p)
    d = p - t
    ad = np.abs(d)
    huber = np.mean(np.where(ad < 1.0, 0.5 * d * d, ad - 0.5))
    num = 0.0
    cnt = 0
    step = 512
    for i0 in range(0, n, step):
        i1 = min(i0 + step, n)
        pd = p[i0:i1, None] - p[None, :]
        td = t[i0:i1, None] - t[None, :]
        sign = np.where(td > 0, 1.0, -1.0)
        idx = np.arange(n)
        mask = (td != 0) & (idx[i0:i1, None] < idx[None, :])
        hinge = np.maximum(0.0, 1.0 - sign * pd)
        num += hinge[mask].sum()
        cnt += int(mask.sum())
    ranking = num / float(np.float32(cnt)) if cnt > 0 else 0.0
    return np.float32(ALPHA * huber + BETA * ranking)


def kernel(predictions: np.ndarray, targets: np.ndarray) -> np.ndarray:
    predictions = np.asarray(predictions, dtype=np.float32)
    targets = np.asarray(targets, dtype=np.float32)

    if len(np.unique(targets)) > 16 or predictions.shape[0] < NCORES * P:
        return np.array(_host_fallback(predictions, targets), dtype=np.float32)

    meta, bcols, cp_cores, _ps = _plan(targets, predictions)
    nc = _build_program(_shape_key(meta))
    in_maps = _make_inputs(meta, bcols, cp_cores, predictions, targets)
    res = run_bass_kernel_spmd(nc, in_maps, list(range(NCORES)))
    return np.array(_gather(meta, cp_cores, res.results), dtype=np.float32)
